# revision 30
# baseline (speedup 1.0000x reference)
"""MLA attention kernel for Trainium2, sharded over 8 NeuronCores.

Sharding: core = (batch b in {0,1}) x (kv-group g in {0..3}).
Each core handles one batch's 4 query heads + 1 kv head of one group and
produces a partial output projection [T, DIM] (fp16); the host sums the 4
group partials per batch in fp32.

v2 design notes (per core):
  - All matmul inputs fp16, accumulation fp32 in PSUM.
  - Q/K produced token-major for norm+rope (free-dim reductions), then
    moved to head-dim-major [d, t] via DMA XBAR transposes (keeps the PE
    free for real matmuls and HAM-warm).
  - ACT engine uses only {Exp, Ln, Copy, Square} = one table set
    (natural_log_exp_and_others) -> zero ACT_TABLE_LOAD swaps.
    rstd = exp(-0.5*ln(ms+eps)) batched per 512-token chunk.
  - Softmax: S.T tiles computed in [128, 1024] PSUM groups (2 kk-tiles per
    exp/accumulate op) to amortize the 352-cycle ACT op overhead.
  - P row-sum accumulated in fp16 on DVE into a 2-slot accumulator,
    reduced by 2 accumulating ones-matmuls, reciprocal on DVE,
    partition-broadcast on GpSimd.
  - Warmup matmuls at kernel start keep the PE HAM clock-gate warm while
    the first weight/activation DMAs land.
"""

import os
from contextlib import ExitStack

import numpy as np
import ml_dtypes

import concourse.bass as bass
import concourse.bass_isa as bass_isa
import concourse.bacc as bacc
import concourse.tile as tile
from concourse import mybir
from concourse.bass_utils import run_bass_kernel_spmd
from concourse.masks import make_identity

BF16 = np.float16
NH, NKV, HD, RANK, DIM = 16, 4, 128, 512, 2048
B, T = 2, 2048
NT = T // 128          # 16 token tiles
NCH = T // 512         # 4 token chunks
KD = DIM // 128        # 16 contraction tiles over model dim
KR = RANK // 128       # 4 contraction tiles over rank
EPS = 1.1920928955078125e-07
SCALE = 1.0 / float(np.sqrt(HD))
EXP_BIAS = -2.0794415416798357  # -ln(8): keeps exp outputs inside fp16 range; cancels in normalize

F32 = mybir.dt.float32
BF = mybir.dt.float16   # fp16: same PE throughput as bf16, 8x the mantissa
AF = mybir.ActivationFunctionType
AX = mybir.AxisListType
ALU = mybir.AluOpType


def _rope_tables():
    inv_freq = 1.0 / (10000.0 ** (np.arange(0, HD, 2, dtype=np.float64) / HD))
    t = np.arange(T, dtype=np.float64)
    f = np.outer(t, inv_freq)                      # [T, 64]
    cos, sin = np.cos(f), np.sin(f)
    coscat = np.concatenate([cos, cos], axis=1)    # [T, 128]
    sincat = np.concatenate([sin, -sin], axis=1)   # [T, 128]
    return coscat.astype(np.float32), sincat.astype(np.float32)


def _build_module():
    nc = bacc.Bacc("TRN2", target_bir_lowering=False, debug=False)

    # DRAM tensors are laid out exactly as their SBUF tiles (partition-major)
    # so every load is one fully-contiguous DMA.
    xt = nc.dram_tensor("xt", [NCH, 128, KD, 512], BF, kind="ExternalInput").ap()
    wq = nc.dram_tensor("wq", [128, KD, 512], BF, kind="ExternalInput").ap()
    wkv = nc.dram_tensor("wkv", [128, KD, 256], BF, kind="ExternalInput").ap()
    wpj = nc.dram_tensor("wpj", [128, 4, 2048], BF, kind="ExternalInput").ap()
    cosd = nc.dram_tensor("cosd", [128, NT, 128], BF, kind="ExternalInput").ap()
    sind = nc.dram_tensor("sind", [128, NT, 128], BF, kind="ExternalInput").ap()
    gain = nc.dram_tensor("gain", [128, 4], F32, kind="ExternalInput").ap()
    out = nc.dram_tensor("out", [T, DIM], BF, kind="ExternalOutput").ap()

    with tile.TileContext(nc) as tc:
        with ExitStack() as ctx:
            _emit(ctx, tc, out, xt, wq, wkv, wpj, cosd, sind, gain)

    # Constrain the ACT table-set chooser to natural_log_exp_and_others
    # (contains exp+ln+square+copy+identity = every ACT func this kernel
    # uses) so a single ACT_TABLE_LOAD is emitted instead of ping-ponging
    # between per-function default sets. Positional set ids are preserved.
    def _one_set_table_loads():
        import bass_rust as _br
        from concourse.hw_specs import get_activation_tables
        tables = []
        for name, funcs in get_activation_tables(nc.m.arch).items():
            if name == "natural_log_exp_and_others":
                tables.append((name, funcs))
            else:
                tables.append((name, set()))
        _br.insert_act_table_loads(nc, tables)

    nc.insert_act_table_loads = _one_set_table_loads
    nc.compile()
    return nc


def _emit(ctx, tc, out, xt, wq, wkv, wpj, cosd, sind, gain):
    nc = tc.nc

    persist = ctx.enter_context(tc.tile_pool(name="persist", bufs=1))
    kt_sb = persist.tile([128, T], BF)             # K.T  [d, t]
    kv_sb = persist.tile([128, NT, 2, 128], BF)    # K,V token-major per tile
    trimask_t = persist.tile([128, 128], F32)      # keep where tk <= tq
    ones128 = persist.tile([128, 128], BF)         # rowsum+broadcast matmul
    eps_sb = persist.tile([128, 1], F32)
    gain_sb = persist.tile([128, 4], F32)
    cos_sb = persist.tile([128, NT, 128], BF)
    sin_sb = persist.tile([128, NT, 128], BF)
    expb_sb = persist.tile([128, 1], F32)
    ident = persist.tile([128, 128], BF)
    negident = persist.tile([128, 128], BF)
    tri01 = persist.tile([128, 128], BF)
    wu_lhs = persist.tile([128, 128], BF)
    wu_rhs = persist.tile([128, 512], BF)
    make_identity(nc, ident)
    # mask-as-matmul operands: negident.T @ tri01 = -30000 where tk > tq.
    # (-30000 fits fp16; exp(SCALE*(s-30000)) == 0 in fp32.)
    nc.scalar.activation(out=negident, in_=ident, func=AF.Copy, scale=-30000.0)
    nc.gpsimd.memset(tri01, 1.0)
    nc.gpsimd.affine_select(
        out=tri01, in_=tri01,
        compare_op=mybir.AluOpType.is_ge, fill=0.0,
        base=-1, pattern=[[-1, 128]], channel_multiplier=1,
    )

    # transposed causal mask for S.T tiles [tk, tq]: keep x<=y, else -1e9
    nc.gpsimd.memset(trimask_t, 0.0)
    nc.gpsimd.affine_select(
        out=trimask_t, in_=trimask_t,
        compare_op=mybir.AluOpType.is_ge, fill=-1e9,
        base=0, pattern=[[1, 128]], channel_multiplier=-1,
    )
    nc.vector.memset(ones128, 1.0)
    nc.vector.memset(eps_sb, EPS)
    nc.vector.memset(expb_sb, EXP_BIAS)
    nc.vector.memset(wu_lhs, 0.0)
    nc.vector.memset(wu_rhs, 0.0)

    xtp = ctx.enter_context(tc.tile_pool(name="xtp", bufs=2))
    w1 = ctx.enter_context(tc.tile_pool(name="w1", bufs=1))
    qtp = ctx.enter_context(tc.tile_pool(name="qtp", bufs=2))
    otp = ctx.enter_context(tc.tile_pool(name="otp", bufs=2))
    qfp = ctx.enter_context(tc.tile_pool(name="qfp", bufs=8))
    scp = ctx.enter_context(tc.tile_pool(name="scp", bufs=3))
    ssp = ctx.enter_context(tc.tile_pool(name="ssp", bufs=2))
    ptp = ctx.enter_context(tc.tile_pool(name="ptp", bufs=6))
    paccp = ctx.enter_context(tc.tile_pool(name="paccp", bufs=2))
    sc2 = ctx.enter_context(tc.tile_pool(name="sc2", bufs=2))
    osb = ctx.enter_context(tc.tile_pool(name="osb", bufs=12))
    mmp = ctx.enter_context(tc.tile_pool(name="mm", bufs=2, space="PSUM"))
    spool = ctx.enter_context(tc.tile_pool(name="sps", bufs=2, space="PSUM"))
    opool = ctx.enter_context(tc.tile_pool(name="ops", bufs=1, space="PSUM"))
    trp = ctx.enter_context(tc.tile_pool(name="trp", bufs=1, space="PSUM"))

    # ---- warm up the PE HAM clock gate while the first DMAs land
    for i in range(20):
        wups = mmp.tile([128, 512], F32, tag="mm")
        nc.tensor.matmul(wups, lhsT=wu_lhs, rhs=wu_rhs, start=True, stop=True)

    wq_sb = w1.tile([128, KD, 512], BF)
    wkv_sb = w1.tile([128, KD, 256], BF)
    xtc0 = xtp.tile([128, KD, 512], BF, tag="xtc")
    for k in range(KD):
        nc.sync.dma_start(out=wkv_sb[:, k, :], in_=wkv[:, k, :])
        nc.sync.dma_start(out=xtc0[:, k, :], in_=xt[0, :, k, :])
    for k in range(KD):
        nc.sync.dma_start(out=wq_sb[:, k, :], in_=wq[:, k, :])
    nc.sync.dma_start(out=gain_sb, in_=gain)
    # split the rope tables and proj weights into per-tile pieces so no
    # single DMA queue serializes a 512KB transfer
    for t in range(NT):
        nc.sync.dma_start(out=cos_sb[:, t, :], in_=cosd[:, t, :])
        nc.sync.dma_start(out=sin_sb[:, t, :], in_=sind[:, t, :])
    wpj_sb = w1.tile([128, 4, 2048], BF)
    for h in range(4):
        for n in range(4):
            nc.sync.dma_start(out=wpj_sb[:, h, n * 512:(n + 1) * 512],
                              in_=wpj[:, h, n * 512:(n + 1) * 512])

    def _psum_copy(dst, src):
        # PSUM->SBUF copies gate PSUM pool rotation (and thus the PE MM
        # stream); keep them on DVE where queue latency is lowest.
        nc.vector.tensor_copy(dst, src)

    def emit_kv_tile(c, xtc, ss_k, i):
        # ---- K, V matmuls (Wdown folded into Wkup/Wvup on host) + K sumsq
        t = 4 * c + i
        ps = mmp.tile([128, 2, 128], F32, tag="mm")
        for k in range(KD):
            nc.tensor.matmul(
                ps.rearrange("p a b -> p (a b)"),
                lhsT=xtc[:, k, i * 128:(i + 1) * 128],
                rhs=wkv_sb[:, k, :],
                start=(k == 0),
                stop=(k == KD - 1),
            )
        nc.scalar.copy(kv_sb[:, t, :, :], ps)
        sqk = scp.tile([128, 128], BF, tag="sqk")
        nc.scalar.activation(out=sqk, in_=kv_sb[:, t, 0, :],
                             func=AF.Square)
        nc.vector.tensor_reduce(
            out=ss_k[:, i:i + 1], in_=sqk, axis=AX.X, op=ALU.add
        )

    def emit_q_tile(c, xtc, ss_q, qfs, i):
        # ---- Q matmuls + sumsq for one token tile
        ps = mmp.tile([128, 4, 128], F32, tag="mm")
        for k in range(KD):
            nc.tensor.matmul(
                ps.rearrange("p a b -> p (a b)"),
                lhsT=xtc[:, k, i * 128:(i + 1) * 128],
                rhs=wq_sb[:, k, :],
                start=(k == 0),
                stop=(k == KD - 1),
            )
        qf = qfp.tile([128, 4, 128], BF, tag="qf")
        qfs.append(qf)
        nc.scalar.copy(qf.rearrange("p a b -> p (a b)"),
                       ps.rearrange("p a b -> p (a b)"))
        sq = scp.tile([128, 4, 128], BF, tag="sq")
        nc.scalar.activation(out=sq.rearrange("p a b -> p (a b)"),
                             in_=qf.rearrange("p a b -> p (a b)"),
                             func=AF.Square)
        nc.vector.tensor_reduce(
            out=ss_q[:, i, :], in_=sq, axis=AX.X, op=ALU.add
        )

    def emit_rstd(ss_k, ss_q):
        # ---- batched rstd for the whole chunk: exp(-0.5*ln(ms+eps))
        lnk = ssp.tile([128, 4], F32, tag="lnk")
        rstdk = ssp.tile([128, 4], F32, tag="rstdk")
        nc.scalar.activation(out=lnk, in_=ss_k, func=AF.Ln,
                             bias=eps_sb, scale=1.0 / HD)
        nc.scalar.activation(out=rstdk, in_=lnk, func=AF.Exp, scale=-0.5)
        lnq = ssp.tile([128, 16], F32, tag="lnq")
        rstdq = ssp.tile([128, 4, 4], F32, tag="rstdq")
        nc.scalar.activation(out=lnq,
                             in_=ss_q.rearrange("p a b -> p (a b)"),
                             func=AF.Ln, bias=eps_sb, scale=1.0 / HD)
        nc.scalar.activation(out=rstdq.rearrange("p a b -> p (a b)"),
                             in_=lnq, func=AF.Exp, scale=-0.5)
        nc.vector.tensor_mul(rstdq, rstdq, _bcast_mid(gain_sb, 4))
        return rstdk, rstdq

    def emit_finish_tile(c, i, qt_c, qfs, rstdk, rstdq):
        # ---- normalize + rope + PE-transpose one token tile into [d, t]
        t = 4 * c + i
        nc.vector.tensor_scalar_mul(
            kv_sb[:, t, 0, :], in0=kv_sb[:, t, 0, :],
            scalar1=rstdk[:, i:i + 1],
        )
        kn = scp.tile([128, 1, 128], BF, tag="kn")
        _rope(nc, scp, kn, kv_sb[:, t, 0:1, :],
              cos_sb[:, t, :], sin_sb[:, t, :], 1)
        tpk = trp.tile([128, 4, 128], BF, tag="tr")
        nc.tensor.transpose(tpk[:, 0, :], kn[:, 0, :], ident)
        nc.scalar.copy(kt_sb[:, t * 128:(t + 1) * 128], tpk[:, 0, :])
        qf = qfs[i]
        for h in range(4):
            nc.vector.tensor_scalar_mul(
                qf[:, h, :], in0=qf[:, h, :], scalar1=rstdq[:, i, h:h + 1]
            )
        qn = scp.tile([128, 4, 128], BF, tag="qn")
        _rope(nc, scp, qn, qf, cos_sb[:, t, :], sin_sb[:, t, :], 4)
        tpq = trp.tile([128, 4, 128], BF, tag="tr")
        for h in range(4):
            nc.tensor.transpose(tpq[:, h, :], qn[:, h, :], ident)
        nc.scalar.copy(qt_c[:, :, i * 128:(i + 1) * 128], tpq)

    def emit_attn_head(c, qt_c, ot_c, h):
        last_kk = 4 * c + 3
        if True:
            po = opool.tile([128, 512], F32, tag="o")
            pacc = paccp.tile([128, 2, 512], BF, tag="pacc")
            # groups of 2 kk-tiles: (kk, pt_offset, x0) where x0 is the
            # first valid tq column of that kk tile
            groups = [[(2 * p, 0, 0), (2 * p + 1, 512, 0)] for p in range(2 * c)]
            groups.append([(4 * c, 0, 0), (4 * c + 1, 512, 128)])
            groups.append([(4 * c + 2, 0, 256), (4 * c + 3, 256, 384)])
            def emit_pv(grp, pt):
                for (kk, off, x0) in grp:
                    nc.tensor.matmul(
                        po[:, x0:512],
                        lhsT=kv_sb[:, kk, 1, :],
                        rhs=pt[:, off:off + 512 - x0],
                        start=(kk == 0),
                        stop=(kk == last_kk),
                        skip_group_check=True,
                    )

            pending = None  # defer PV one group so the PE never waits on exp
            for gi, grp in enumerate(groups):
                wtot = sum(512 - x0 for (_, _, x0) in grp)
                st = spool.tile([128, 1024], F32, tag="s")
                for (kk, off, x0) in grp:
                    diag = kk >= 4 * c
                    nc.tensor.matmul(
                        st[:, off:off + 512 - x0],
                        lhsT=kt_sb[:, kk * 128:(kk + 1) * 128],
                        rhs=qt_c[:, h, x0:512],
                        start=True,
                        stop=not diag,
                        skip_group_check=True,
                    )
                    if diag:
                        # add -30000 where tk > tq on the 128-wide diagonal
                        # block, via PE accumulation (keeps S->exp on-chip
                        # path free of a DVE hop)
                        nc.tensor.matmul(
                            st[:, off:off + 128],
                            lhsT=negident,
                            rhs=tri01,
                            start=False,
                            stop=True,
                            skip_group_check=True,
                        )
                pt = ptp.tile([128, 1024], BF, tag="pt")
                nc.scalar.activation(
                    out=pt[:, 0:wtot], in_=st[:, 0:wtot],
                    func=AF.Exp, scale=SCALE, bias=expb_sb,
                )
                # accumulate row sums (over tk) into the 2-slot accumulator;
                # the first group of each head initializes it instead
                if grp[0][2] == 0 and grp[1][2] == 0:
                    pf = pacc.rearrange("p a b -> p (a b)")
                    if gi == 0:
                        nc.vector.tensor_copy(pf, pt[:, 0:1024])
                    else:
                        nc.vector.tensor_add(pf, pf, pt[:, 0:1024])
                else:
                    if gi == 0:   # c == 0: diag group initializes
                        nc.vector.tensor_copy(pacc[:, 0, :], pt[:, 0:512])
                        nc.vector.memset(pacc[:, 1, 0:128], 0.0)
                        nc.vector.tensor_copy(pacc[:, 1, 128:512],
                                              pt[:, 512:896])
                    else:
                        for s, (kk, off, x0) in enumerate(grp):
                            nc.vector.tensor_add(
                                pacc[:, s, x0:512], pacc[:, s, x0:512],
                                pt[:, off:off + 512 - x0],
                            )
                if pending is not None:
                    emit_pv(*pending)
                pending = (grp, pt)
            emit_pv(*pending)
            # drain po to SBUF immediately so the PSUM bank frees for the
            # next head's PV; normalization happens off the critical path
            oraw = sc2.tile([128, 512], BF, tag="oraw")
            nc.vector.tensor_copy(oraw, po)

            def epilogue():
                # P row-sum + partition-broadcast fused in one PE op:
                # all-ones stationary sums pacc across partitions into every
                # out partition. Deferred by the caller so the PE hits these
                # matmuls only after pacc's DVE accumulation has drained.
                rsb = opool.tile([128, 512], F32, tag="o")
                nc.tensor.matmul(rsb, lhsT=ones128, rhs=pacc[:, 0, :],
                                 start=True, stop=False)
                nc.tensor.matmul(rsb, lhsT=ones128, rhs=pacc[:, 1, :],
                                 start=False, stop=True)
                rbc = sc2.tile([128, 512], F32, tag="rbc")
                nc.vector.reciprocal_approx_fast(out=rbc, in_=rsb)
                nc.vector.tensor_mul(ot_c[:, h, :], oraw, rbc)

            return epilogue

    def emit_proj(c, ot_c, tiles=(0, 1, 2, 3)):
        # ---- output projection for this chunk
        for i in tiles:
            t = 4 * c + i
            for n in range(4):
                pj = mmp.tile([128, 512], F32, tag="mm")
                for h in range(4):
                    nc.tensor.matmul(
                        pj,
                        lhsT=ot_c[:, h, i * 128:(i + 1) * 128],
                        rhs=wpj_sb[:, h, n * 512:(n + 1) * 512],
                        start=(h == 0),
                        stop=(h == 3),
                    )
                outsb = osb.tile([128, 512], BF, tag="out")
                if n % 2 == 0:
                    nc.scalar.copy(outsb, pj)
                else:
                    nc.vector.tensor_copy(outsb, pj)
                nc.sync.dma_start(
                    out=out[t * 128:(t + 1) * 128, n * 512:(n + 1) * 512],
                    in_=outsb,
                )

    def emit_warm_mm(n):
        # HAM-visible dummy matmuls to bridge transpose-only PE windows
        for _ in range(n):
            wups = mmp.tile([128, 512], F32, tag="mm")
            nc.tensor.matmul(wups, lhsT=wu_lhs, rhs=wu_rhs,
                             start=True, stop=True)

    qts = {}
    xtcs = {0: xtc0}
    ot_p = None
    for c in range(NCH):
        if c + 1 < NCH:
            # prefetch next chunk's activations one iteration early
            xtn = xtp.tile([128, KD, 512], BF, tag="xtc")
            for k in range(KD):
                nc.sync.dma_start(out=xtn[:, k, :], in_=xt[c + 1, :, k, :])
            xtcs[c + 1] = xtn
        xtc = xtcs[c]
        ss_k = ssp.tile([128, 4], F32, tag="ssk")
        ss_q = ssp.tile([128, 4, 4], F32, tag="ssq")
        qfs = []
        if c >= 1:
            # interleave: each prev-chunk attention head is followed by one
            # kv+q tile pair, so the head's rowsum/reciprocal epilogue and
            # its ACT exp work overlap the kv/q matmul stream
            qt_p = qts[c - 1]
            ot_n = otp.tile([128, 4, 512], BF, tag="ot")
            for i in range(4):
                ep = emit_attn_head(c - 1, qt_p, ot_n, i)
                emit_kv_tile(c, xtc, ss_k, i)
                ep()
                emit_q_tile(c, xtc, ss_q, qfs, i)
            rstdk, rstdq = emit_rstd(ss_k, ss_q)
            qt_c = qtp.tile([128, 4, 512], BF, tag="qt")
            # proj of the prev chunk runs on the PE while the DVE/ACT chain
            # (rstd -> normalize -> rope) prepares this chunk's finishes
            emit_proj(c - 1, ot_n)
            emit_finish_tile(c, 0, qt_c, qfs, rstdk, rstdq)
            emit_finish_tile(c, 1, qt_c, qfs, rstdk, rstdq)
            emit_finish_tile(c, 2, qt_c, qfs, rstdk, rstdq)
            emit_finish_tile(c, 3, qt_c, qfs, rstdk, rstdq)
        else:
            # chunk 0 is DMA-gated: kv tiles need only wkv+xt (first loads),
            # q tiles need wq (lands later)
            for i in range(4):
                emit_kv_tile(0, xtc, ss_k, i)
            for i in range(4):
                emit_q_tile(0, xtc, ss_q, qfs, i)
            rstdk, rstdq = emit_rstd(ss_k, ss_q)
            qt_c = qtp.tile([128, 4, 512], BF, tag="qt")
            for i in range(4):
                emit_finish_tile(0, i, qt_c, qfs, rstdk, rstdq)
                emit_warm_mm(3)
        qts[c] = qt_c
    # tail: the last chunk's attention heads run back-to-back; each head's
    # epilogue is deferred one head so pacc's DVE chain never stalls the PE
    ot_l = otp.tile([128, 4, 512], BF, tag="ot")
    prev_ep = None
    for h in range(4):
        ep = emit_attn_head(NCH - 1, qts[NCH - 1], ot_l, h)
        if prev_ep is not None:
            prev_ep()
        prev_ep = ep
    prev_ep()
    emit_proj(NCH - 1, ot_l)


def _rope(nc, scp, out_t, ps, cos_t, sin_t, nh):
    """out = ps * coscat + swap_halves(ps) * sincat, per head.

    ps: [128, nh, 128] fp16 SBUF, out_t: [128, nh, 128] fp16,
    cos_t/sin_t: [128, 128] fp16 tables (broadcast over the head dim).
    """
    t1 = scp.tile([128, nh, 128], BF, tag=f"ropea{nh}")
    t2 = scp.tile([128, nh, 128], BF, tag=f"ropeb{nh}")
    cos_b = _bcast_mid(cos_t, nh)
    sin_b = _bcast_mid(sin_t, nh)
    nc.vector.tensor_mul(t1, ps, cos_b)
    nc.vector.tensor_mul(t2, _swap_halves(ps), sin_b)
    nc.vector.tensor_add(out_t, t1, t2)


def _bcast_mid(ap2d, nh):
    """[128, 128] -> [128, nh, 128] with 0-stride on the middle dim."""
    if nh == 1:
        return bass.AP(tensor=ap2d.tensor, offset=ap2d.offset,
                       ap=[ap2d.ap[0], [0, 1], ap2d.ap[1]])
    return bass.AP(tensor=ap2d.tensor, offset=ap2d.offset,
                   ap=[ap2d.ap[0], [0, nh], ap2d.ap[1]])


def _swap_halves(ap3d):
    """[128, nh, 128] -> same shape reading cols [64:128, 0:64] of last dim."""
    last = ap3d.ap[-1]
    step = last[0]
    return bass.AP(tensor=ap3d.tensor, offset=ap3d.offset + 64 * step,
                   ap=list(ap3d.ap[:-1]) + [[-64 * step, 2], [step, 64]])


def _ensure_ntff_hook():
    """Install the axon NTFF profiling hook if the image lacks
    antenv.axon_hooks (needed for trace=True under axon)."""
    try:
        from antenv.axon_hooks import get_axon_ntff_profile_hook  # noqa: F401
        return
    except ImportError:
        pass
    import contextlib
    import ctypes
    import sys
    import types

    mod = types.ModuleType("antenv.axon_hooks")
    _state = {"hook": None}
    mod.set_axon_ntff_profile_hook = lambda h: _state.update(hook=h)
    mod.get_axon_ntff_profile_hook = lambda: _state["hook"]
    import antenv

    sys.modules["antenv.axon_hooks"] = mod
    antenv.axon_hooks = mod

    so_path = "/opt/axon/libaxon_pjrt.so"
    if not os.path.exists(so_path):
        return
    lib = ctypes.CDLL(so_path)
    if not hasattr(lib, "axon_start_nrt_profile"):
        return
    lib.axon_start_nrt_profile.argtypes = [
        ctypes.POINTER(ctypes.c_int64),
        ctypes.c_size_t,
    ]
    lib.axon_start_nrt_profile.restype = ctypes.c_int64
    lib.axon_stop_nrt_profile.argtypes = [ctypes.c_char_p]
    lib.axon_stop_nrt_profile.restype = ctypes.c_int64

    @contextlib.contextmanager
    def _hook(output_dir, device_ids):
        import jax

        jax.devices()
        if device_ids:
            ids = (ctypes.c_int64 * len(device_ids))(*device_ids)
            rc = lib.axon_start_nrt_profile(ids, len(device_ids))
        else:
            rc = lib.axon_start_nrt_profile(None, 0)
        if rc != 0:
            raise RuntimeError(f"axon_start_nrt_profile rc={rc}")
        try:
            yield
        finally:
            n = lib.axon_stop_nrt_profile(str(output_dir).encode())
            if n < 0:
                raise RuntimeError(f"axon_stop_nrt_profile rc={n}")
            print(f"profile: {n} file(s) written to {output_dir}")

    mod.set_axon_ntff_profile_hook(_hook)


_NC_CACHE = None


def _get_module():
    global _NC_CACHE
    if _NC_CACHE is None:
        _NC_CACHE = _build_module()
    return _NC_CACHE


def _prep_core_inputs(x, Wq, Wdown, Wkup, Wvup, Wproj, q_gain, b, g):
    coscat, sincat = _rope_tables()
    xb = x[b].astype(BF16)                                   # [T, DIM]
    xt = np.ascontiguousarray(
        xb.reshape(NCH, 512, KD, 128).transpose(0, 3, 2, 1)
    )                                                        # [4,128,16,512]
    wqg = Wq[g * 512:(g + 1) * 512].astype(BF16)             # [512, 2048]
    wq = np.ascontiguousarray(wqg.reshape(512, KD, 128).transpose(2, 1, 0))
    # fold Wdown into the kv up-projections (associativity):
    # k_g = lat @ Wkup_g.T = x @ (Wkup_g @ Wdown).T
    wd64 = Wdown.astype(np.float64)
    wk_eff = (Wkup[g * 128:(g + 1) * 128].astype(np.float64) @ wd64)
    wv_eff = (Wvup[g * 128:(g + 1) * 128].astype(np.float64) @ wd64)
    weff = np.concatenate([wk_eff, wv_eff], axis=0).astype(BF16)  # [256, 2048]
    wkv = np.ascontiguousarray(weff.reshape(256, KD, 128).transpose(2, 1, 0))
    wpg = Wproj[:, g * 512:(g + 1) * 512].astype(BF16)       # [2048, 512]
    wpj = np.ascontiguousarray(wpg.reshape(2048, 4, 128).transpose(2, 1, 0))
    cos = np.ascontiguousarray(
        coscat.astype(BF16).reshape(NT, 128, 128).transpose(1, 0, 2)
    )
    sin = np.ascontiguousarray(
        sincat.astype(BF16).reshape(NT, 128, 128).transpose(1, 0, 2)
    )
    gain = np.ascontiguousarray(
        np.broadcast_to(q_gain[g * 4:(g + 1) * 4].astype(np.float32), (128, 4))
    )
    return {
        "xt": xt, "wq": wq, "wkv": wkv,
        "wpj": wpj, "cosd": cos, "sind": sin, "gain": gain,
    }


def kernel(x, Wq, Wdown, Wkup, Wvup, Wproj, q_gain, _trace=False):
    x = np.asarray(x, dtype=np.float32)
    nc = _get_module()
    in_maps = []
    for core in range(8):
        b, g = divmod(core, 4)
        in_maps.append(
            _prep_core_inputs(x, np.asarray(Wq), np.asarray(Wdown),
                              np.asarray(Wkup), np.asarray(Wvup),
                              np.asarray(Wproj), np.asarray(q_gain), b, g)
        )
    if _trace:
        _ensure_ntff_hook()
    res = run_bass_kernel_spmd(nc, in_maps, core_ids=list(range(8)),
                               trace=_trace)
    outs = [np.asarray(r["out"], dtype=np.float32) for r in res.results]
    y = np.empty((B, T, DIM), dtype=np.float32)
    for b in range(B):
        y[b] = outs[4 * b + 0] + outs[4 * b + 1] + outs[4 * b + 2] + outs[4 * b + 3]
    kernel._last_results = res
    return y



# revision 31
# speedup vs baseline: 1.0547x; 1.0547x over previous
"""MLA attention kernel for Trainium2, sharded over 8 NeuronCores.

Sharding: core = (batch b in {0,1}) x (kv-group g in {0..3}).
Each core handles one batch's 4 query heads + 1 kv head of one group and
produces a partial output projection [T, DIM] (fp16); the host sums the 4
group partials per batch in fp32.

v2 design notes (per core):
  - All matmul inputs fp16, accumulation fp32 in PSUM.
  - Q/K produced token-major for norm+rope (free-dim reductions), then
    moved to head-dim-major [d, t] via DMA XBAR transposes (keeps the PE
    free for real matmuls and HAM-warm).
  - ACT engine uses only {Exp, Ln, Copy, Square} = one table set
    (natural_log_exp_and_others) -> zero ACT_TABLE_LOAD swaps.
    rstd = exp(-0.5*ln(ms+eps)) batched per 512-token chunk.
  - Softmax: S.T tiles computed in [128, 1024] PSUM groups (2 kk-tiles per
    exp/accumulate op) to amortize the 352-cycle ACT op overhead.
  - P row-sum accumulated in fp16 on DVE into a 2-slot accumulator,
    reduced by 2 accumulating ones-matmuls, reciprocal on DVE,
    partition-broadcast on GpSimd.
  - Warmup matmuls at kernel start keep the PE HAM clock-gate warm while
    the first weight/activation DMAs land.
"""

import os
from contextlib import ExitStack

import numpy as np
import ml_dtypes

import concourse.bass as bass
import concourse.bass_isa as bass_isa
import concourse.bacc as bacc
import concourse.tile as tile
from concourse import mybir
from concourse.bass_utils import run_bass_kernel_spmd
from concourse.masks import make_identity

BF16 = np.float16
NH, NKV, HD, RANK, DIM = 16, 4, 128, 512, 2048
B, T = 2, 2048
NT = T // 128          # 16 token tiles
NCH = T // 512         # 4 token chunks
KD = DIM // 128        # 16 contraction tiles over model dim
KR = RANK // 128       # 4 contraction tiles over rank
EPS = 1.1920928955078125e-07
SCALE = 1.0 / float(np.sqrt(HD))
EXP_BIAS = -2.0794415416798357  # -ln(8): keeps exp outputs inside fp16 range; cancels in normalize

F32 = mybir.dt.float32
BF = mybir.dt.float16   # fp16: same PE throughput as bf16, 8x the mantissa
AF = mybir.ActivationFunctionType
AX = mybir.AxisListType
ALU = mybir.AluOpType


def _rope_tables():
    inv_freq = 1.0 / (10000.0 ** (np.arange(0, HD, 2, dtype=np.float64) / HD))
    t = np.arange(T, dtype=np.float64)
    f = np.outer(t, inv_freq)                      # [T, 64]
    cos, sin = np.cos(f), np.sin(f)
    coscat = np.concatenate([cos, cos], axis=1)    # [T, 128]
    sincat = np.concatenate([sin, -sin], axis=1)   # [T, 128]
    return coscat.astype(np.float32), sincat.astype(np.float32)


def _build_module():
    nc = bacc.Bacc("TRN2", target_bir_lowering=False, debug=False)

    # DRAM tensors are laid out exactly as their SBUF tiles (partition-major)
    # so every load is one fully-contiguous DMA.
    xt = nc.dram_tensor("xt", [NCH, 128, KD, 512], BF, kind="ExternalInput").ap()
    wq = nc.dram_tensor("wq", [128, KD, 512], BF, kind="ExternalInput").ap()
    wkv = nc.dram_tensor("wkv", [128, KD, 256], BF, kind="ExternalInput").ap()
    wpj = nc.dram_tensor("wpj", [128, 4, 2048], BF, kind="ExternalInput").ap()
    cosd = nc.dram_tensor("cosd", [128, NT, 128], BF, kind="ExternalInput").ap()
    sind = nc.dram_tensor("sind", [128, NT, 128], BF, kind="ExternalInput").ap()
    gain = nc.dram_tensor("gain", [128, 4], F32, kind="ExternalInput").ap()
    out = nc.dram_tensor("out", [T, DIM], BF, kind="ExternalOutput").ap()

    with tile.TileContext(nc) as tc:
        with ExitStack() as ctx:
            _emit(ctx, tc, out, xt, wq, wkv, wpj, cosd, sind, gain)

    # Constrain the ACT table-set chooser to natural_log_exp_and_others
    # (contains exp+ln+square+copy+identity = every ACT func this kernel
    # uses) so a single ACT_TABLE_LOAD is emitted instead of ping-ponging
    # between per-function default sets. Positional set ids are preserved.
    def _one_set_table_loads():
        import bass_rust as _br
        from concourse.hw_specs import get_activation_tables
        tables = []
        for name, funcs in get_activation_tables(nc.m.arch).items():
            if name == "natural_log_exp_and_others":
                tables.append((name, funcs))
            else:
                tables.append((name, set()))
        _br.insert_act_table_loads(nc, tables)

    nc.insert_act_table_loads = _one_set_table_loads
    nc.compile()
    return nc


def _emit(ctx, tc, out, xt, wq, wkv, wpj, cosd, sind, gain):
    nc = tc.nc

    persist = ctx.enter_context(tc.tile_pool(name="persist", bufs=1))
    kt_sb = persist.tile([128, T], BF)             # K.T  [d, t]
    kv_sb = persist.tile([128, NT, 2, 128], BF)    # K,V token-major per tile
    trimask_t = persist.tile([128, 128], F32)      # keep where tk <= tq
    ones128 = persist.tile([128, 128], BF)         # rowsum+broadcast matmul
    eps_sb = persist.tile([128, 1], F32)
    gain_sb = persist.tile([128, 4], F32)
    cos_sb = persist.tile([128, NT, 128], BF)
    sin_sb = persist.tile([128, NT, 128], BF)
    expb_sb = persist.tile([128, 1], F32)
    ident = persist.tile([128, 128], BF)
    negident = persist.tile([128, 128], BF)
    tri01 = persist.tile([128, 128], BF)
    wu_lhs = persist.tile([128, 128], BF)
    wu_rhs = persist.tile([128, 512], BF)
    make_identity(nc, ident)
    # mask-as-matmul operands: negident.T @ tri01 = -30000 where tk > tq.
    # (-30000 fits fp16; exp(SCALE*(s-30000)) == 0 in fp32.)
    nc.scalar.activation(out=negident, in_=ident, func=AF.Copy, scale=-30000.0)
    nc.gpsimd.memset(tri01, 1.0)
    nc.gpsimd.affine_select(
        out=tri01, in_=tri01,
        compare_op=mybir.AluOpType.is_ge, fill=0.0,
        base=-1, pattern=[[-1, 128]], channel_multiplier=1,
    )

    # transposed causal mask for S.T tiles [tk, tq]: keep x<=y, else -1e9
    nc.gpsimd.memset(trimask_t, 0.0)
    nc.gpsimd.affine_select(
        out=trimask_t, in_=trimask_t,
        compare_op=mybir.AluOpType.is_ge, fill=-1e9,
        base=0, pattern=[[1, 128]], channel_multiplier=-1,
    )
    nc.vector.memset(ones128, 1.0)
    nc.vector.memset(eps_sb, EPS)
    nc.vector.memset(expb_sb, EXP_BIAS)
    nc.vector.memset(wu_lhs, 0.0)
    nc.vector.memset(wu_rhs, 0.0)

    xtp = ctx.enter_context(tc.tile_pool(name="xtp", bufs=2))
    w1 = ctx.enter_context(tc.tile_pool(name="w1", bufs=1))
    qtp = ctx.enter_context(tc.tile_pool(name="qtp", bufs=2))
    otp = ctx.enter_context(tc.tile_pool(name="otp", bufs=2))
    qfp = ctx.enter_context(tc.tile_pool(name="qfp", bufs=8))
    scp = ctx.enter_context(tc.tile_pool(name="scp", bufs=3))
    ssp = ctx.enter_context(tc.tile_pool(name="ssp", bufs=2))
    ptp = ctx.enter_context(tc.tile_pool(name="ptp", bufs=6))
    paccp = ctx.enter_context(tc.tile_pool(name="paccp", bufs=2))
    sc2 = ctx.enter_context(tc.tile_pool(name="sc2", bufs=2))
    osb = ctx.enter_context(tc.tile_pool(name="osb", bufs=12))
    mmp = ctx.enter_context(tc.tile_pool(name="mm", bufs=2, space="PSUM"))
    spool = ctx.enter_context(tc.tile_pool(name="sps", bufs=2, space="PSUM"))
    opool = ctx.enter_context(tc.tile_pool(name="ops", bufs=1, space="PSUM"))
    trp = ctx.enter_context(tc.tile_pool(name="trp", bufs=1, space="PSUM"))

    # ---- warm up the PE HAM clock gate while the first DMAs land
    for i in range(20):
        wups = mmp.tile([128, 512], F32, tag="mm")
        nc.tensor.matmul(wups, lhsT=wu_lhs, rhs=wu_rhs, start=True, stop=True)

    wq_sb = w1.tile([128, KD, 512], BF)
    wkv_sb = w1.tile([128, KD, 256], BF)
    xtc0 = xtp.tile([128, KD, 512], BF, tag="xtc")
    for k in range(KD):
        nc.sync.dma_start(out=wkv_sb[:, k, :], in_=wkv[:, k, :])
        nc.sync.dma_start(out=xtc0[:, k, :], in_=xt[0, :, k, :])
    for k in range(KD):
        nc.sync.dma_start(out=wq_sb[:, k, :], in_=wq[:, k, :])
    nc.sync.dma_start(out=gain_sb, in_=gain)
    # split the rope tables into pieces so no single DMA queue serializes
    # a 512KB transfer (cos/sin are needed by chunk 0's finish tiles)
    for t in range(0, NT, 4):
        nc.sync.dma_start(out=cos_sb[:, t:t + 4, :], in_=cosd[:, t:t + 4, :])
        nc.sync.dma_start(out=sin_sb[:, t:t + 4, :], in_=sind[:, t:t + 4, :])
    wpj_sb = w1.tile([128, 4, 2048], BF)
    for h in range(4):
        nc.sync.dma_start(out=wpj_sb[:, h, :], in_=wpj[:, h, :])

    def _psum_copy(dst, src):
        # PSUM->SBUF copies gate PSUM pool rotation (and thus the PE MM
        # stream); keep them on DVE where queue latency is lowest.
        nc.vector.tensor_copy(dst, src)

    def emit_kv_tile(c, xtc, ss_k, i):
        # ---- K, V matmuls (Wdown folded into Wkup/Wvup on host) + K sumsq
        t = 4 * c + i
        ps = mmp.tile([128, 2, 128], F32, tag="mm")
        for k in range(KD):
            nc.tensor.matmul(
                ps.rearrange("p a b -> p (a b)"),
                lhsT=xtc[:, k, i * 128:(i + 1) * 128],
                rhs=wkv_sb[:, k, :],
                start=(k == 0),
                stop=(k == KD - 1),
            )
        nc.scalar.copy(kv_sb[:, t, :, :], ps)
        sqk = scp.tile([128, 128], BF, tag="sqk")
        nc.scalar.activation(out=sqk, in_=kv_sb[:, t, 0, :],
                             func=AF.Square)
        nc.vector.tensor_reduce(
            out=ss_k[:, i:i + 1], in_=sqk, axis=AX.X, op=ALU.add
        )

    def emit_q_tile(c, xtc, ss_q, qfs, i):
        # ---- Q matmuls + sumsq for one token tile
        ps = mmp.tile([128, 4, 128], F32, tag="mm")
        for k in range(KD):
            nc.tensor.matmul(
                ps.rearrange("p a b -> p (a b)"),
                lhsT=xtc[:, k, i * 128:(i + 1) * 128],
                rhs=wq_sb[:, k, :],
                start=(k == 0),
                stop=(k == KD - 1),
            )
        qf = qfp.tile([128, 4, 128], BF, tag="qf")
        qfs.append(qf)
        nc.scalar.copy(qf.rearrange("p a b -> p (a b)"),
                       ps.rearrange("p a b -> p (a b)"))
        sq = scp.tile([128, 4, 128], BF, tag="sq")
        nc.scalar.activation(out=sq.rearrange("p a b -> p (a b)"),
                             in_=qf.rearrange("p a b -> p (a b)"),
                             func=AF.Square)
        nc.vector.tensor_reduce(
            out=ss_q[:, i, :], in_=sq, axis=AX.X, op=ALU.add
        )

    def emit_rstd(ss_k, ss_q):
        # ---- batched rstd for the whole chunk: exp(-0.5*ln(ms+eps))
        lnk = ssp.tile([128, 4], F32, tag="lnk")
        rstdk = ssp.tile([128, 4], F32, tag="rstdk")
        nc.scalar.activation(out=lnk, in_=ss_k, func=AF.Ln,
                             bias=eps_sb, scale=1.0 / HD)
        nc.scalar.activation(out=rstdk, in_=lnk, func=AF.Exp, scale=-0.5)
        lnq = ssp.tile([128, 16], F32, tag="lnq")
        rstdq = ssp.tile([128, 4, 4], F32, tag="rstdq")
        nc.scalar.activation(out=lnq,
                             in_=ss_q.rearrange("p a b -> p (a b)"),
                             func=AF.Ln, bias=eps_sb, scale=1.0 / HD)
        nc.scalar.activation(out=rstdq.rearrange("p a b -> p (a b)"),
                             in_=lnq, func=AF.Exp, scale=-0.5)
        nc.vector.tensor_mul(rstdq, rstdq, _bcast_mid(gain_sb, 4))
        return rstdk, rstdq

    def emit_finish_tile(c, i, qt_c, qfs, rstdk, rstdq):
        # ---- normalize + rope + PE-transpose one token tile into [d, t]
        t = 4 * c + i
        nc.vector.tensor_scalar_mul(
            kv_sb[:, t, 0, :], in0=kv_sb[:, t, 0, :],
            scalar1=rstdk[:, i:i + 1],
        )
        kn = scp.tile([128, 1, 128], BF, tag="kn")
        _rope(nc, scp, kn, kv_sb[:, t, 0:1, :],
              cos_sb[:, t, :], sin_sb[:, t, :], 1)
        tpk = trp.tile([128, 4, 128], BF, tag="tr")
        nc.tensor.transpose(tpk[:, 0, :], kn[:, 0, :], ident)
        nc.scalar.copy(kt_sb[:, t * 128:(t + 1) * 128], tpk[:, 0, :])
        qf = qfs[i]
        for h in range(4):
            nc.vector.tensor_scalar_mul(
                qf[:, h, :], in0=qf[:, h, :], scalar1=rstdq[:, i, h:h + 1]
            )
        qn = scp.tile([128, 4, 128], BF, tag="qn")
        _rope(nc, scp, qn, qf, cos_sb[:, t, :], sin_sb[:, t, :], 4)
        tpq = trp.tile([128, 4, 128], BF, tag="tr")
        for h in range(4):
            nc.tensor.transpose(tpq[:, h, :], qn[:, h, :], ident)
        nc.scalar.copy(qt_c[:, :, i * 128:(i + 1) * 128], tpq)

    def emit_attn_head(c, qt_c, ot_c, h):
        last_kk = 4 * c + 3
        if True:
            po = opool.tile([128, 512], F32, tag="o")
            pacc = paccp.tile([128, 2, 512], BF, tag="pacc")
            # groups of 2 kk-tiles: (kk, pt_offset, x0) where x0 is the
            # first valid tq column of that kk tile
            groups = [[(2 * p, 0, 0), (2 * p + 1, 512, 0)] for p in range(2 * c)]
            groups.append([(4 * c, 0, 0), (4 * c + 1, 512, 128)])
            groups.append([(4 * c + 2, 0, 256), (4 * c + 3, 256, 384)])
            def emit_pv(grp, pt):
                for (kk, off, x0) in grp:
                    nc.tensor.matmul(
                        po[:, x0:512],
                        lhsT=kv_sb[:, kk, 1, :],
                        rhs=pt[:, off:off + 512 - x0],
                        start=(kk == 0),
                        stop=(kk == last_kk),
                        skip_group_check=True,
                    )

            pending = None  # defer PV one group so the PE never waits on exp
            for gi, grp in enumerate(groups):
                wtot = sum(512 - x0 for (_, _, x0) in grp)
                st = spool.tile([128, 1024], F32, tag="s")
                for (kk, off, x0) in grp:
                    diag = kk >= 4 * c
                    nc.tensor.matmul(
                        st[:, off:off + 512 - x0],
                        lhsT=kt_sb[:, kk * 128:(kk + 1) * 128],
                        rhs=qt_c[:, h, x0:512],
                        start=True,
                        stop=not diag,
                        skip_group_check=True,
                    )
                    if diag:
                        # add -30000 where tk > tq on the 128-wide diagonal
                        # block, via PE accumulation (keeps S->exp on-chip
                        # path free of a DVE hop)
                        nc.tensor.matmul(
                            st[:, off:off + 128],
                            lhsT=negident,
                            rhs=tri01,
                            start=False,
                            stop=True,
                            skip_group_check=True,
                        )
                pt = ptp.tile([128, 1024], BF, tag="pt")
                nc.scalar.activation(
                    out=pt[:, 0:wtot], in_=st[:, 0:wtot],
                    func=AF.Exp, scale=SCALE, bias=expb_sb,
                )
                # accumulate row sums (over tk) into the 2-slot accumulator;
                # the first group of each head initializes it instead
                if grp[0][2] == 0 and grp[1][2] == 0:
                    pf = pacc.rearrange("p a b -> p (a b)")
                    if gi == 0:
                        nc.vector.tensor_copy(pf, pt[:, 0:1024])
                    else:
                        nc.vector.tensor_add(pf, pf, pt[:, 0:1024])
                else:
                    if gi == 0:   # c == 0: diag group initializes
                        nc.vector.tensor_copy(pacc[:, 0, :], pt[:, 0:512])
                        nc.vector.memset(pacc[:, 1, 0:128], 0.0)
                        nc.vector.tensor_copy(pacc[:, 1, 128:512],
                                              pt[:, 512:896])
                    else:
                        for s, (kk, off, x0) in enumerate(grp):
                            nc.vector.tensor_add(
                                pacc[:, s, x0:512], pacc[:, s, x0:512],
                                pt[:, off:off + 512 - x0],
                            )
                if pending is not None:
                    emit_pv(*pending)
                pending = (grp, pt)
            emit_pv(*pending)
            # drain po to SBUF immediately so the PSUM bank frees for the
            # next head's PV; normalization happens off the critical path
            oraw = sc2.tile([128, 512], BF, tag="oraw")
            nc.vector.tensor_copy(oraw, po)

            def epilogue():
                # P row-sum + partition-broadcast fused in one PE op:
                # all-ones stationary sums pacc across partitions into every
                # out partition. Deferred by the caller so the PE hits these
                # matmuls only after pacc's DVE accumulation has drained.
                rsb = opool.tile([128, 512], F32, tag="o")
                nc.tensor.matmul(rsb, lhsT=ones128, rhs=pacc[:, 0, :],
                                 start=True, stop=False)
                nc.tensor.matmul(rsb, lhsT=ones128, rhs=pacc[:, 1, :],
                                 start=False, stop=True)
                rbc = sc2.tile([128, 512], F32, tag="rbc")
                nc.vector.reciprocal_approx_fast(out=rbc, in_=rsb)
                nc.vector.tensor_mul(ot_c[:, h, :], oraw, rbc)

            return epilogue

    def emit_proj(c, ot_c, tiles=(0, 1, 2, 3)):
        # ---- output projection for this chunk
        for i in tiles:
            t = 4 * c + i
            for n in range(4):
                pj = mmp.tile([128, 512], F32, tag="mm")
                for h in range(4):
                    nc.tensor.matmul(
                        pj,
                        lhsT=ot_c[:, h, i * 128:(i + 1) * 128],
                        rhs=wpj_sb[:, h, n * 512:(n + 1) * 512],
                        start=(h == 0),
                        stop=(h == 3),
                    )
                outsb = osb.tile([128, 512], BF, tag="out")
                if n % 2 == 0:
                    nc.scalar.copy(outsb, pj)
                else:
                    nc.vector.tensor_copy(outsb, pj)
                nc.sync.dma_start(
                    out=out[t * 128:(t + 1) * 128, n * 512:(n + 1) * 512],
                    in_=outsb,
                )

    def emit_warm_mm(n):
        # HAM-visible dummy matmuls to bridge transpose-only PE windows
        for _ in range(n):
            wups = mmp.tile([128, 512], F32, tag="mm")
            nc.tensor.matmul(wups, lhsT=wu_lhs, rhs=wu_rhs,
                             start=True, stop=True)

    qts = {}
    xtcs = {0: xtc0}
    ot_p = None
    for c in range(NCH):
        if c + 1 < NCH:
            # prefetch next chunk's activations one iteration early
            xtn = xtp.tile([128, KD, 512], BF, tag="xtc")
            for k in range(KD):
                nc.sync.dma_start(out=xtn[:, k, :], in_=xt[c + 1, :, k, :])
            xtcs[c + 1] = xtn
        xtc = xtcs[c]
        ss_k = ssp.tile([128, 4], F32, tag="ssk")
        ss_q = ssp.tile([128, 4, 4], F32, tag="ssq")
        qfs = []
        if c >= 1:
            # interleave: each prev-chunk attention head is followed by one
            # kv+q tile pair, so the head's rowsum/reciprocal epilogue and
            # its ACT exp work overlap the kv/q matmul stream
            qt_p = qts[c - 1]
            ot_n = otp.tile([128, 4, 512], BF, tag="ot")
            for i in range(4):
                ep = emit_attn_head(c - 1, qt_p, ot_n, i)
                emit_kv_tile(c, xtc, ss_k, i)
                ep()
                emit_q_tile(c, xtc, ss_q, qfs, i)
            rstdk, rstdq = emit_rstd(ss_k, ss_q)
            qt_c = qtp.tile([128, 4, 512], BF, tag="qt")
            # proj of the prev chunk runs on the PE while the DVE/ACT chain
            # (rstd -> normalize -> rope) prepares this chunk's finishes
            emit_proj(c - 1, ot_n)
            emit_finish_tile(c, 0, qt_c, qfs, rstdk, rstdq)
            emit_finish_tile(c, 1, qt_c, qfs, rstdk, rstdq)
            emit_finish_tile(c, 2, qt_c, qfs, rstdk, rstdq)
            emit_finish_tile(c, 3, qt_c, qfs, rstdk, rstdq)
        else:
            # chunk 0 is DMA-gated: kv tiles need only wkv+xt (first loads),
            # q tiles need wq (lands later)
            for i in range(4):
                emit_kv_tile(0, xtc, ss_k, i)
            for i in range(4):
                emit_q_tile(0, xtc, ss_q, qfs, i)
            rstdk, rstdq = emit_rstd(ss_k, ss_q)
            qt_c = qtp.tile([128, 4, 512], BF, tag="qt")
            for i in range(4):
                emit_finish_tile(0, i, qt_c, qfs, rstdk, rstdq)
                emit_warm_mm(3)
        qts[c] = qt_c
    # tail: the last chunk's attention heads run back-to-back; each head's
    # epilogue is deferred one head so pacc's DVE chain never stalls the PE
    ot_l = otp.tile([128, 4, 512], BF, tag="ot")
    prev_ep = None
    for h in range(4):
        ep = emit_attn_head(NCH - 1, qts[NCH - 1], ot_l, h)
        if prev_ep is not None:
            prev_ep()
        prev_ep = ep
    prev_ep()
    emit_proj(NCH - 1, ot_l)


def _rope(nc, scp, out_t, ps, cos_t, sin_t, nh):
    """out = ps * coscat + swap_halves(ps) * sincat, per head.

    ps: [128, nh, 128] fp16 SBUF, out_t: [128, nh, 128] fp16,
    cos_t/sin_t: [128, 128] fp16 tables (broadcast over the head dim).
    """
    t1 = scp.tile([128, nh, 128], BF, tag=f"ropea{nh}")
    t2 = scp.tile([128, nh, 128], BF, tag=f"ropeb{nh}")
    cos_b = _bcast_mid(cos_t, nh)
    sin_b = _bcast_mid(sin_t, nh)
    nc.vector.tensor_mul(t1, ps, cos_b)
    nc.vector.tensor_mul(t2, _swap_halves(ps), sin_b)
    nc.vector.tensor_add(out_t, t1, t2)


def _bcast_mid(ap2d, nh):
    """[128, 128] -> [128, nh, 128] with 0-stride on the middle dim."""
    if nh == 1:
        return bass.AP(tensor=ap2d.tensor, offset=ap2d.offset,
                       ap=[ap2d.ap[0], [0, 1], ap2d.ap[1]])
    return bass.AP(tensor=ap2d.tensor, offset=ap2d.offset,
                   ap=[ap2d.ap[0], [0, nh], ap2d.ap[1]])


def _swap_halves(ap3d):
    """[128, nh, 128] -> same shape reading cols [64:128, 0:64] of last dim."""
    last = ap3d.ap[-1]
    step = last[0]
    return bass.AP(tensor=ap3d.tensor, offset=ap3d.offset + 64 * step,
                   ap=list(ap3d.ap[:-1]) + [[-64 * step, 2], [step, 64]])


def _ensure_ntff_hook():
    """Install the axon NTFF profiling hook if the image lacks
    antenv.axon_hooks (needed for trace=True under axon)."""
    try:
        from antenv.axon_hooks import get_axon_ntff_profile_hook  # noqa: F401
        return
    except ImportError:
        pass
    import contextlib
    import ctypes
    import sys
    import types

    mod = types.ModuleType("antenv.axon_hooks")
    _state = {"hook": None}
    mod.set_axon_ntff_profile_hook = lambda h: _state.update(hook=h)
    mod.get_axon_ntff_profile_hook = lambda: _state["hook"]
    import antenv

    sys.modules["antenv.axon_hooks"] = mod
    antenv.axon_hooks = mod

    so_path = "/opt/axon/libaxon_pjrt.so"
    if not os.path.exists(so_path):
        return
    lib = ctypes.CDLL(so_path)
    if not hasattr(lib, "axon_start_nrt_profile"):
        return
    lib.axon_start_nrt_profile.argtypes = [
        ctypes.POINTER(ctypes.c_int64),
        ctypes.c_size_t,
    ]
    lib.axon_start_nrt_profile.restype = ctypes.c_int64
    lib.axon_stop_nrt_profile.argtypes = [ctypes.c_char_p]
    lib.axon_stop_nrt_profile.restype = ctypes.c_int64

    @contextlib.contextmanager
    def _hook(output_dir, device_ids):
        import jax

        jax.devices()
        if device_ids:
            ids = (ctypes.c_int64 * len(device_ids))(*device_ids)
            rc = lib.axon_start_nrt_profile(ids, len(device_ids))
        else:
            rc = lib.axon_start_nrt_profile(None, 0)
        if rc != 0:
            raise RuntimeError(f"axon_start_nrt_profile rc={rc}")
        try:
            yield
        finally:
            n = lib.axon_stop_nrt_profile(str(output_dir).encode())
            if n < 0:
                raise RuntimeError(f"axon_stop_nrt_profile rc={n}")
            print(f"profile: {n} file(s) written to {output_dir}")

    mod.set_axon_ntff_profile_hook(_hook)


_NC_CACHE = None


def _get_module():
    global _NC_CACHE
    if _NC_CACHE is None:
        _NC_CACHE = _build_module()
    return _NC_CACHE


def _prep_core_inputs(x, Wq, Wdown, Wkup, Wvup, Wproj, q_gain, b, g):
    coscat, sincat = _rope_tables()
    xb = x[b].astype(BF16)                                   # [T, DIM]
    xt = np.ascontiguousarray(
        xb.reshape(NCH, 512, KD, 128).transpose(0, 3, 2, 1)
    )                                                        # [4,128,16,512]
    wqg = Wq[g * 512:(g + 1) * 512].astype(BF16)             # [512, 2048]
    wq = np.ascontiguousarray(wqg.reshape(512, KD, 128).transpose(2, 1, 0))
    # fold Wdown into the kv up-projections (associativity):
    # k_g = lat @ Wkup_g.T = x @ (Wkup_g @ Wdown).T
    wd64 = Wdown.astype(np.float64)
    wk_eff = (Wkup[g * 128:(g + 1) * 128].astype(np.float64) @ wd64)
    wv_eff = (Wvup[g * 128:(g + 1) * 128].astype(np.float64) @ wd64)
    weff = np.concatenate([wk_eff, wv_eff], axis=0).astype(BF16)  # [256, 2048]
    wkv = np.ascontiguousarray(weff.reshape(256, KD, 128).transpose(2, 1, 0))
    wpg = Wproj[:, g * 512:(g + 1) * 512].astype(BF16)       # [2048, 512]
    wpj = np.ascontiguousarray(wpg.reshape(2048, 4, 128).transpose(2, 1, 0))
    cos = np.ascontiguousarray(
        coscat.astype(BF16).reshape(NT, 128, 128).transpose(1, 0, 2)
    )
    sin = np.ascontiguousarray(
        sincat.astype(BF16).reshape(NT, 128, 128).transpose(1, 0, 2)
    )
    gain = np.ascontiguousarray(
        np.broadcast_to(q_gain[g * 4:(g + 1) * 4].astype(np.float32), (128, 4))
    )
    return {
        "xt": xt, "wq": wq, "wkv": wkv,
        "wpj": wpj, "cosd": cos, "sind": sin, "gain": gain,
    }


def kernel(x, Wq, Wdown, Wkup, Wvup, Wproj, q_gain, _trace=False):
    x = np.asarray(x, dtype=np.float32)
    nc = _get_module()
    in_maps = []
    for core in range(8):
        b, g = divmod(core, 4)
        in_maps.append(
            _prep_core_inputs(x, np.asarray(Wq), np.asarray(Wdown),
                              np.asarray(Wkup), np.asarray(Wvup),
                              np.asarray(Wproj), np.asarray(q_gain), b, g)
        )
    if _trace:
        _ensure_ntff_hook()
    res = run_bass_kernel_spmd(nc, in_maps, core_ids=list(range(8)),
                               trace=_trace)
    outs = [np.asarray(r["out"], dtype=np.float32) for r in res.results]
    y = np.empty((B, T, DIM), dtype=np.float32)
    for b in range(B):
        y[b] = outs[4 * b + 0] + outs[4 * b + 1] + outs[4 * b + 2] + outs[4 * b + 3]
    kernel._last_results = res
    return y

